# revision 1
# baseline (speedup 1.0000x reference)
"""Kernel-target-alignment loss on 8 TRN2 NeuronCores.

Math: Xs = X*sqrt(params); d2_ij = ||Xs_i - Xs_j||^2; K = exp(-d2) (diag == 1);
kta = sum(K*tt^T) / (N*sqrt(sum(K*K)));  return -kta.

Strategy (row-sharded across 8 cores, 1024 rows each):
  A_ij = 2*sum_d p_d x_i x_j - sq_i - sq_j  (= -d2), built per [128,1024] tile as
  one augmented fp32 matmul (K=65: [2p*X^T ; ones] x [X^T ; -sq]) giving
  2G - sq_j in PSUM; the -sq_i row term rides the ACT exp bias (a [128,1] column
  obtained by a K=1 PE matmul "transpose" of the -sq row, so row and column sq
  are bit-identical).  E = exp(A) in bf16.  Two fused DVE tensor_tensor_reduce
  ops per tile give row-sums of E*E (for sum K^2) and E*t_col (for t^T K t).
  Per-core partials return to the host for the final scalar combine.  No sqrt
  anywhere (lhs carries p*x, rhs carries x), so A_ii is fp32-exact ~0 and
  bf16(exp(A_ii)) == 1.0 exactly, matching the reference's unit diagonal.
"""

import numpy as np

import concourse.bass as bass
import concourse.bacc as bacc
import concourse.tile as tile
import concourse.mybir as mybir
from concourse.bass_utils import run_bass_kernel_spmd

N = 8192
D = 64
NCORES = 8
RPC = N // NCORES          # 1024 rows per core
NRB = RPC // 128           # 8 row blocks of 128 rows
CW = 1024                  # column tile width (2 PSUM banks fp32)
NCT = N // CW              # 8 column tiles
NSLOT = NRB * NCT          # 64 accumulator slots per core

F32 = mybir.dt.float32
BF16 = mybir.dt.bfloat16


def _ap(tensor, ap):
    return bass.AP(tensor=tensor, offset=0, ap=ap)


def build_kernel(variant="stt"):
    nc = bacc.Bacc("TRN2", target_bir_lowering=False)

    xt_d = nc.dram_tensor("xt", [D, N], F32, kind="ExternalInput")
    lt_d = nc.dram_tensor("lt", [D, RPC], F32, kind="ExternalInput")
    t_d = nc.dram_tensor("t", [N], F32, kind="ExternalInput")
    params_d = nc.dram_tensor("params", [D], F32, kind="ExternalInput")
    rsq_d = nc.dram_tensor("rsq_scratch", [RPC], F32)
    s1o_d = nc.dram_tensor("s1o", [128, NSLOT], F32, kind="ExternalOutput")
    s2o_d = nc.dram_tensor("s2o", [128, NSLOT], F32, kind="ExternalOutput")

    with tile.TileContext(nc) as tc:
        with (
            tc.tile_pool(name="const", bufs=1) as cpool,
            tc.tile_pool(name="ztiles", bufs=3) as zpool,
            tc.tile_pool(name="etile", bufs=4) as epool,
            tc.tile_pool(name="scratch", bufs=4) as spool,
            tc.tile_pool(name="mmpsum", bufs=2, space="PSUM") as mpool,
            tc.tile_pool(name="setpsum", bufs=3, space="PSUM") as qpool,
        ):
            # ---- persistent SBUF tensors -------------------------------------
            R = cpool.tile([D + 1, N], F32, tag="R")       # [x^T ; -sq]
            L = cpool.tile([D + 1, RPC], F32, tag="L")     # local [2p*x^T ; ones]
            lt_sb = cpool.tile([D, RPC], F32, tag="ltsb")  # local x^T slice
            sqloc = cpool.tile([1, RPC], F32, tag="sqloc")  # local -sq row
            xs1l = cpool.tile([D, RPC], F32, tag="xs1l")   # local p*x^T
            xs1 = cpool.tile([D, N], F32, tag="xs1")       # p * x^T
            tcol = cpool.tile([128, N], BF16, tag="tcol")  # t broadcast to 128 parts
            tcolf = cpool.tile([128, N], F32, tag="tcolf")
            psb = cpool.tile([D, 1], F32, tag="psb")
            p2sb = cpool.tile([D, 1], F32, tag="p2sb")
            neg1 = cpool.tile([D, 1], F32, tag="neg1")
            rsqn = cpool.tile([128, NRB], F32, tag="rsqn")
            rsqn2 = cpool.tile([128, NRB], F32, tag="rsqn2")
            s1acc = cpool.tile([128, NSLOT], F32, tag="s1acc")
            s2acc = cpool.tile([128, NSLOT], F32, tag="s2acc")

            # ---- setup -------------------------------------------------------
            for s in range(16):
                sl = slice(s * 512, (s + 1) * 512)
                nc.sync.dma_start(out=R[0:D, sl], in_=xt_d[:, sl])
            nc.gpsimd.dma_start(out=psb[:, :], in_=_ap(params_d, [[1, D], [0, 1]]))
            nc.sync.dma_start(out=lt_sb[:, :], in_=lt_d[:, :])
            for s in range(8):
                sl = slice(s * (N // 8), (s + 1) * (N // 8))
                nc.sync.dma_start(
                    out=tcolf[:, sl],
                    in_=bass.AP(tensor=t_d, offset=s * (N // 8), ap=[[0, 128], [1, N // 8]]),
                )
            nc.vector.tensor_scalar_mul(p2sb[:, :], psb[:, :], 2.0)
            nc.vector.memset(neg1[:, :], -1.0)
            nc.gpsimd.memset(L[D : D + 1, :], 1.0)
            nc.vector.tensor_scalar_mul(L[0:D, :], lt_sb[:, :], p2sb[:, :])
            nc.vector.tensor_scalar_mul(xs1l[:, :], lt_sb[:, :], psb[:, :])

            # xs1 = p*x^T  (sliced for pipelining)
            for s in range(8):
                sl = slice(s * (N // 8), (s + 1) * (N // 8))
                nc.vector.tensor_scalar_mul(xs1[:, sl], R[0:D, sl], psb[:, :])

            # col-layout -sq (R row D) via PE partition-reduce of z = xs1 * x
            for s in range(16):
                sl = slice(s * 512, (s + 1) * 512)
                zt = zpool.tile([D, 512], F32, tag="z")
                nc.vector.tensor_mul(zt[:, :], xs1[:, sl], R[0:D, sl])
                q = qpool.tile([128, 512], F32, tag="qps")
                nc.tensor.matmul(
                    q[0:1, :], neg1[:, :], zt[:, :], start=True, stop=True
                )
                nc.scalar.copy(out=R[D : D + 1, sl], in_=q[0:1, :])

            # local -sq row for this core's rows (same fp ops as column path)
            for s in range(RPC // 512):
                sl = slice(s * 512, (s + 1) * 512)
                zt = zpool.tile([D, 512], F32, tag="z")
                nc.vector.tensor_mul(zt[:, :], xs1l[:, sl], lt_sb[:, sl])
                q = qpool.tile([128, 512], F32, tag="qps")
                nc.tensor.matmul(
                    q[0:1, :], neg1[:, :], zt[:, :], start=True, stop=True
                )
                nc.scalar.copy(out=sqloc[:, sl], in_=q[0:1, :])

            # row-layout -sq for the exp bias: bounce through DRAM so the
            # [1, RPC] row can be re-read as a [128, NRB] partition-major tile:
            # rsqn[p, rb] = sqloc[0, rb*128 + p]
            nc.gpsimd.dma_start(out=_ap(rsq_d, [[0, 1], [1, RPC]]), in_=sqloc[:, :])
            nc.gpsimd.dma_start(out=rsqn[:, :], in_=_ap(rsq_d, [[1, 128], [128, NRB]]))
            nc.vector.tensor_scalar_mul(rsqn2[:, :], rsqn[:, :], 2.0)

            # tcol: cast broadcast t to bf16
            for s in range(8):
                sl = slice(s * (N // 8), (s + 1) * (N // 8))
                nc.vector.tensor_copy(out=tcol[:, sl], in_=tcolf[:, sl])

            if variant == "nott":
                nc.vector.memset(s1acc[:, :], 0.0)
                nc.vector.memset(s2acc[:, :], 0.0)
            # ---- main loop ---------------------------------------------------
            for rb in range(NRB):
                lhsT = L[:, rb * 128 : (rb + 1) * 128]
                bias = rsqn[:, rb : rb + 1]
                for ct in range(NCT):
                    slot = rb * NCT + ct
                    mm = mpool.tile([128, CW], F32, tag="mm")
                    for j in range(CW // 512):
                        sl = slice(ct * CW + j * 512, ct * CW + (j + 1) * 512)
                        nc.tensor.matmul(
                            mm[:, j * 512 : (j + 1) * 512],
                            lhsT,
                            R[:, sl],
                            start=True,
                            stop=True,
                        )
                    EDT = F32 if variant == "ttrf32" else BF16
                    E = epool.tile([128, CW], EDT, tag="E")
                    if variant == "noexp":
                        nc.scalar.copy(out=E[:, :], in_=mm[:, :])
                    else:
                        nc.scalar.activation(
                            out=E[:, :], in_=mm[:, :],
                            func=mybir.ActivationFunctionType.Exp,
                            bias=bias, scale=1.0,
                        )
                    if variant == "nott":
                        continue
                    sc1 = spool.tile([128, CW], EDT, tag="sc1")
                    tcol_in = tcolf if variant == "ttrf32" else tcol
                    if variant in ("stt", "g1", "act2"):
                        if variant == "g1":
                            nc.gpsimd.scalar_tensor_tensor(
                                out=sc1[:, :], in0=E[:, :], scalar=1.0, in1=E[:, :],
                                op0=mybir.AluOpType.mult, op1=mybir.AluOpType.mult,
                                accum_out=s1acc[:, slot : slot + 1],
                            )
                        elif variant == "act2":
                            nc.scalar.activation(
                                out=sc1[:, :], in_=mm[:, :],
                                func=mybir.ActivationFunctionType.Exp,
                                bias=rsqn2[:, rb : rb + 1], scale=2.0,
                                accum_out=s1acc[:, slot : slot + 1],
                            )
                        else:
                            nc.vector.scalar_tensor_tensor(
                                out=sc1[:, :], in0=E[:, :], scalar=1.0, in1=E[:, :],
                                op0=mybir.AluOpType.mult, op1=mybir.AluOpType.mult,
                                accum_out=s1acc[:, slot : slot + 1],
                            )
                        sc2 = spool.tile([128, CW], EDT, tag="sc2")
                        nc.vector.scalar_tensor_tensor(
                            out=sc2[:, :], in0=E[:, :], scalar=1.0,
                            in1=tcol_in[:, ct * CW : (ct + 1) * CW],
                            op0=mybir.AluOpType.mult, op1=mybir.AluOpType.mult,
                            accum_out=s2acc[:, slot : slot + 1],
                        )
                    else:
                        nc.vector.tensor_tensor_reduce(
                            out=sc1[:, :], in0=E[:, :], in1=E[:, :],
                            scale=1.0, scalar=0.0,
                            op0=mybir.AluOpType.mult, op1=mybir.AluOpType.add,
                            accum_out=s1acc[:, slot : slot + 1],
                        )
                        sc2 = spool.tile([128, CW], EDT, tag="sc2")
                        nc.vector.tensor_tensor_reduce(
                            out=sc2[:, :], in0=E[:, :],
                            in1=tcol_in[:, ct * CW : (ct + 1) * CW],
                            scale=1.0, scalar=0.0,
                            op0=mybir.AluOpType.mult, op1=mybir.AluOpType.add,
                            accum_out=s2acc[:, slot : slot + 1],
                        )

            nc.sync.dma_start(out=s1o_d[:, :], in_=s1acc[:, :])
            nc.sync.dma_start(out=s2o_d[:, :], in_=s2acc[:, :])

    nc.compile()
    return nc


_NC_CACHE = None


def make_in_maps(X, target, params):
    X = np.ascontiguousarray(X, dtype=np.float32)
    target = np.ascontiguousarray(target, dtype=np.float32)
    params = np.ascontiguousarray(params, dtype=np.float32)
    xt = np.ascontiguousarray(X.T)
    return [
        {
            "xt": xt,
            "lt": np.ascontiguousarray(xt[:, c * RPC : (c + 1) * RPC]),
            "t": target,
            "params": params,
        }
        for c in range(NCORES)
    ]


def kernel(X, target, params):
    global _NC_CACHE
    X = np.ascontiguousarray(X, dtype=np.float32)
    target = np.ascontiguousarray(target, dtype=np.float32)
    params = np.ascontiguousarray(params, dtype=np.float32)

    in_maps = make_in_maps(X, target, params)

    if _NC_CACHE is None:
        _NC_CACHE = build_kernel()
    res = run_bass_kernel_spmd(_NC_CACHE, in_maps, core_ids=list(range(NCORES)))

    s1 = 0.0
    s2 = 0.0
    for c in range(NCORES):
        s1o = res.results[c]["s1o"]  # [128, NSLOT]
        s2o = res.results[c]["s2o"]  # [128, NSLOT]
        s1 += float(s1o.sum())
        u = s2o.reshape(128, NRB, NCT).sum(axis=2)              # [128, NRB]
        tb = target[c * RPC : (c + 1) * RPC].reshape(NRB, 128)  # [NRB, 128]
        s2 += float(np.sum(u.T * tb))

    val = -s2 / (N * np.sqrt(s1))
    return np.array(val, dtype=np.float32)



# revision 2
# speedup vs baseline: 1.0422x; 1.0422x over previous
"""KTA loss v2c: symmetric circulant sharding, s2 on PE chains, exact diag.

Same sharding as v2 (see kernel_v2.py docstring). Key points:
- E = exp(2 Xs_i.Xs_j - sqb16_j + bias_i) in bf16, persisted in SBUF.
- bias_i = -(2 sq_i + nsqb_i) is host-computed so the diagonal entry is
  exp(~1e-6) -> exactly 1.0 in bf16: no on-device diagonal correction at all.
  Host adds sum(t^2) - sum(tb16*t) to s2 to swap the computed diagonal for
  the exact one; s1 needs no adjustment (1.0^2 both ways).
- s1 = sum E^2 via DVE STT accum (32 weighted slots).
- s2 = t^T E t via PE: per 512-col position, a PSUM [1,512] chain over the
  covering strips, stationary = w*t_strip (block weight folded in, bf16);
  rows are copied PSUM->SBUF (ACT/DVE split), bounced through DRAM into a
  [16,512] tile, and reduced against exact-f32 t in one STT.
"""

import numpy as np
import ml_dtypes

import concourse.bass as bass
import concourse.bacc as bacc
import concourse.tile as tile
import concourse.mybir as mybir
from concourse.bass_utils import run_bass_kernel_spmd

N = 8192
D = 64
NCORES = 8
NG = 16
GW = 512
NSTRIP = 8

F32 = mybir.dt.float32
BF16 = mybir.dt.bfloat16
BF = ml_dtypes.bfloat16

ACT_CHUNKS_A = [(0, 1536), (1536, 1536), (3072, 1536)]
ACT_CHUNKS_B = [(4096, 1536), (5632, 1536), (7168, 1024)]
RED_A = [(0, 512, 1.0), (512, 1024, 2.0), (1536, 1536, 2.0), (3072, 1536, 2.0)]
RED_B = [(4096, 512, 1.0), (4608, 1024, 2.0), (5632, 1536, 2.0),
         (7168, 1024, 2.0)]
NRED_A = len(RED_A)
NRED_B = len(RED_B)
NSLOT = 4 * NRED_A + 4 * NRED_B      # 32 s1 slots
ESTRIDE = 4608
# acc columns: [0,32) s1, 32 = s2-chain (partitions 0..9), [33,45) = B-strip
# s2-STT slots (3 pieces x 4 strips), [45,49) = A-strip pos-8 s2-STT slots
ACCW = 56
MOVED_PIECES = [(5120, 512), (5632, 1536), (7168, 1024)]
TCOLP0 = 4096                         # tcolp covers [4096,8192)
NACT_COPIES = 16                      # v-row copies on ACT; rest on DVE
MOVED0 = 5120                         # local col where DVE-handled s2 starts
MOVEDW = 8192 - MOVED0                # 3584 cols, weight 2 blocks only


def _act_chunks(s):
    return ACT_CHUNKS_A if s < 4 else ACT_CHUNKS_B


def _reds(s):
    return RED_A if s < 4 else RED_B


def _slot_base(s):
    return s * NRED_A if s < 4 else 4 * NRED_A + (s - 4) * NRED_B


def _ebase(s):
    return s * ESTRIDE - (0 if s < 4 else 4096)


def build_kernel():
    nc = bacc.Bacc("TRN2", target_bir_lowering=False)

    rt_d = nc.dram_tensor("rt", [D + 1, N], BF16, kind="ExternalInput")
    lt_d = nc.dram_tensor("lt", [D + 1, 1024], BF16, kind="ExternalInput")
    rsqn_d = nc.dram_tensor("rsqn", [128, 2 * NSTRIP], F32, kind="ExternalInput")
    tw_d = nc.dram_tensor("tw", [128, 2 * NSTRIP], BF16, kind="ExternalInput")
    w16_d = nc.dram_tensor("w16", [16, GW], F32, kind="ExternalInput")
    trow_d = nc.dram_tensor("trow", [1, 8192 - TCOLP0], BF16, kind="ExternalInput")
    acc_d = nc.dram_tensor("acc", [128, ACCW], F32, kind="ExternalOutput")
    vs_d = nc.dram_tensor("vs_scratch", [16, GW], F32)

    with tile.TileContext(nc) as tc:
        with (
            tc.tile_pool(name="const", bufs=1) as cpool,
            tc.tile_pool(name="dmyv", bufs=2) as vpool,
            tc.tile_pool(name="mmpool", bufs=2, space="PSUM") as mpool,
            tc.tile_pool(name="trc", bufs=2, space="PSUM") as tpool,
        ):
            R = cpool.tile([D + 1, N], BF16, tag="R")
            L = cpool.tile([D + 1, 1024], BF16, tag="L")
            rsqn = cpool.tile([128, 2 * NSTRIP], F32, tag="rsqn")
            tw = cpool.tile([128, 2 * NSTRIP], BF16, tag="tw")
            w16 = cpool.tile([16, GW], F32, tag="w16")
            vsb = cpool.tile([16, GW], F32, tag="vsb")
            trow = cpool.tile([1, 8192 - TCOLP0], BF16, tag="trow")
            tcolp = cpool.tile([128, 8192 - TCOLP0], BF16, tag="tcolp")
            acc = cpool.tile([128, ACCW], F32, tag="acc")
            Ebuf = cpool.tile([128, NSTRIP * ESTRIDE], BF16, tag="Ebuf")

            # ---- setup: DMAs only, spread across issue queues --------------
            dmaengines = [nc.sync, nc.scalar, nc.sync, nc.gpsimd]
            nc.gpsimd.dma_start(out=L[:, :], in_=lt_d[:, :])
            nc.gpsimd.dma_start(out=rsqn[:, :], in_=rsqn_d[:, :])
            nc.gpsimd.dma_start(out=tw[:, :], in_=tw_d[:, :])
            nc.gpsimd.dma_start(out=w16[:, :], in_=w16_d[:, :])
            nc.gpsimd.dma_start(out=trow[:, :], in_=trow_d[:, :])
            nc.sync.dma_start(out=R[:, 0:512], in_=rt_d[:, 0:512])
            nc.scalar.dma_start(out=R[:, 512:1024], in_=rt_d[:, 512:1024])
            for s8 in range(1, 8):
                sl = slice(s8 * 1024, (s8 + 1) * 1024)
                dmaengines[s8 % 4].dma_start(out=R[:, sl], in_=rt_d[:, sl])
            # Pool builds the column-t tile for the DVE-handled s2 ranges
            for z in range(4):
                zsl = slice(z * 1024, (z + 1) * 1024)
                nc.gpsimd.partition_broadcast(tcolp[:, zsl], trow[:, zsl])

            # ---- strip pass: mm -> exp -> s1 STT on DVE --------------------
            def do_strip(s):
                lhsT = L[:, s * 128:(s + 1) * 128]
                biasap = rsqn[:, s:s + 1]
                sbase = _slot_base(s)
                reds = _reds(s)
                eb = _ebase(s)
                ri = 0
                for c0, cw in _act_chunks(s):
                    mm = mpool.tile([128, 1536], F32, tag="mm")
                    for j in range(cw // 512):
                        nc.tensor.matmul(
                            mm[:, j * 512:(j + 1) * 512],
                            lhsT,
                            R[:, c0 + j * 512:c0 + (j + 1) * 512],
                            start=True, stop=True,
                        )
                    nc.scalar.activation(
                        out=Ebuf[:, eb + c0:eb + c0 + cw], in_=mm[:, 0:cw],
                        func=mybir.ActivationFunctionType.Exp,
                        bias=biasap, scale=1.0,
                    )
                    while ri < len(reds) and \
                            reds[ri][0] + reds[ri][1] <= c0 + cw:
                        r0, rw, _w = reds[ri]
                        slot = sbase + ri
                        sc1 = vpool.tile([128, 1536], BF16, tag="sc1")
                        nc.vector.scalar_tensor_tensor(
                            out=sc1[:, 0:rw],
                            in0=Ebuf[:, eb + r0:eb + r0 + rw],
                            scalar=1.0,
                            in1=Ebuf[:, eb + r0:eb + r0 + rw],
                            op0=mybir.AluOpType.mult,
                            op1=mybir.AluOpType.mult,
                            accum_out=acc[:, slot:slot + 1],
                        )
                        ri += 1
                    if s >= 4 and c0 + cw > MOVED0:
                        # moved-s2 piece of this chunk on DVE (weight 2)
                        p0 = max(c0, MOVED0)
                        pw = c0 + cw - p0
                        pi = MOVED_PIECES.index((p0, pw))
                        sc2 = vpool.tile([128, 1536], BF16, tag="sc2")
                        nc.vector.scalar_tensor_tensor(
                            out=sc2[:, 0:pw],
                            in0=Ebuf[:, eb + p0:eb + p0 + pw],
                            scalar=1.0,
                            in1=tcolp[:, p0 - MOVED0:p0 - MOVED0 + pw],
                            op0=mybir.AluOpType.mult,
                            op1=mybir.AluOpType.mult,
                            accum_out=acc[:, 33 + 3 * (s - 4) + pi:
                                          34 + 3 * (s - 4) + pi],
                        )

            # ---- s2 position chains on PE ----------------------------------
            def covering(pos):
                out = []
                if pos <= 8:
                    for s in range(4):
                        out.append((s, 2 * s if pos == 0 else 2 * s + 1))
                if pos in (8, 9):
                    for s in range(4, 8):
                        out.append((s, 2 * s if pos == 8 else 2 * s + 1))
                return out

            ncopies = [0]

            def do_pos(pos, vstage):
                v = tpool.tile([1, GW], F32, tag="v")
                cov = covering(pos)
                for k, (s, twcol) in enumerate(cov):
                    e0 = _ebase(s) + pos * GW
                    nc.tensor.matmul(
                        v[:, :],
                        tw[:, twcol:twcol + 1],
                        Ebuf[:, e0:e0 + GW],
                        start=(k == 0), stop=(k == len(cov) - 1),
                    )
                # PSUM -> SBUF (split ACT/DVE), then through DRAM onto
                # partition `pos` of vsb
                if ncopies[0] < NACT_COPIES:
                    nc.scalar.copy(out=vstage[:, :], in_=v[:, :])
                else:
                    nc.vector.tensor_copy(out=vstage[:, :], in_=v[:, :])
                dmae = [nc.sync, nc.gpsimd][ncopies[0] % 2]
                ncopies[0] += 1
                dmae.dma_start(
                    out=bass.AP(tensor=vs_d, offset=pos * GW,
                                ap=[[0, 1], [1, GW]]),
                    in_=vstage[:, :])

            vstages = [cpool.tile([1, GW], F32, tag=f"vst{i}", name=f"vst{i}")
                       for i in range(4)]
            for s in range(4):
                do_strip(s)
            for i in range(4):
                do_strip(4 + i)
                do_pos(2 * i, vstages[(2 * i) % 4])
                do_pos(2 * i + 1, vstages[(2 * i + 1) % 4])
            do_pos(8, vstages[0])
            do_pos(9, vstages[1])

            # one gather DMA for the 9 chained position rows
            nc.gpsimd.dma_start(
                out=vsb[0:10, :],
                in_=bass.AP(tensor=vs_d, offset=0, ap=[[GW, 10], [1, GW]]))
            scf = vpool.tile([16, GW], F32, tag="scf")
            nc.vector.scalar_tensor_tensor(
                out=scf[0:10, :], in0=vsb[0:10, :], scalar=1.0, in1=w16[0:10, :],
                op0=mybir.AluOpType.mult, op1=mybir.AluOpType.mult,
                accum_out=acc[0:10, 32:33],
            )

            nc.sync.dma_start(out=acc_d[:, :], in_=acc[:, :])

    nc.compile()
    return nc


_NC_CACHE = None


def _prep(X, target, params):
    Xs = X.astype(np.float64) * np.sqrt(params.astype(np.float64))
    Xsb = Xs.astype(BF)
    sq = (Xsb.astype(np.float64) ** 2).sum(axis=1)
    tb = target.astype(BF)
    return Xsb, sq, tb


def _perm(c):
    return np.concatenate(
        [np.arange(((c + g) % NG) * GW, ((c + g) % NG) * GW + GW)
         for g in range(NG)])


def make_in_maps(X, target, params):
    X = np.asarray(X, dtype=np.float32)
    target = np.asarray(target, dtype=np.float32)
    params = np.asarray(params, dtype=np.float32)
    Xsb, sq, tb = _prep(X, target, params)
    XsbT = np.ascontiguousarray(Xsb.T)
    L2T = np.ascontiguousarray((2.0 * Xsb.astype(np.float64)).astype(BF).T)
    nsqb = (-sq).astype(BF)
    nsqb64 = nsqb.astype(np.float64)
    # bias_i = -(2 sq_i + nsqb_i): diagonal mm + bias == ~0 -> E_ii == 1.0
    bias = -(2.0 * sq + nsqb64)
    in_maps = []
    for c in range(NCORES):
        perm = _perm(c)
        rows = np.concatenate([perm[:GW], perm[8 * GW:9 * GW]])
        rt = np.vstack([XsbT[:, perm], nsqb[perm][None, :]])
        lt = np.vstack([L2T[:, rows], np.ones((1, 1024), dtype=BF)])
        b1 = bias[rows].astype(np.float32).reshape(NSTRIP, 128).T
        b2 = (2.0 * bias[rows]).astype(np.float32).reshape(NSTRIP, 128).T
        rsqn = np.ascontiguousarray(np.hstack([b1, b2]))
        tstrips = tb[rows].reshape(NSTRIP, 128)
        tw = np.zeros((128, 2 * NSTRIP), dtype=BF)
        for s in range(NSTRIP):
            tw[:, 2 * s] = tstrips[s]
            tw[:, 2 * s + 1] = (2.0 * tstrips[s].astype(np.float64)).astype(BF)
        w16 = np.ascontiguousarray(
            target[perm].astype(np.float32).reshape(16, GW))
        trow = np.ascontiguousarray(tb[perm][TCOLP0:8192][None, :])
        in_maps.append({
            "rt": np.ascontiguousarray(rt),
            "lt": np.ascontiguousarray(lt),
            "rsqn": rsqn,
            "tw": tw,
            "w16": w16,
            "trow": trow,
        })
    return in_maps


def combine(results, X, target, params):
    target = np.asarray(target, np.float32)
    t64 = target.astype(np.float64)
    tb64 = target.astype(BF).astype(np.float64)
    s1 = 0.0
    s2 = 0.0
    for c in range(NCORES):
        acc = results[c]["acc"].astype(np.float64)
        for s in range(NSTRIP):
            sbase = _slot_base(s)
            for ri, (_r0, _rw, w) in enumerate(_reds(s)):
                s1 += w * acc[:, sbase + ri].sum()
        s2 += acc[0:10, 32].sum()
        perm = _perm(c)
        rows = np.concatenate([perm[:GW], perm[8 * GW:9 * GW]])
        trows = t64[rows].reshape(NSTRIP, 128)
        for s in range(4, NSTRIP):
            for pi in range(3):
                s2 += 2.0 * (trows[s] * acc[:, 33 + 3 * (s - 4) + pi]).sum()

    # swap computed diagonal (E_ii == 1.0, tb16_i * t_i) for the exact one
    s2 += (t64 * t64).sum() - (tb64 * t64).sum()
    return np.float32(-s2 / (N * np.sqrt(s1)))


def kernel(X, target, params):
    global _NC_CACHE
    X = np.asarray(X, dtype=np.float32)
    target = np.asarray(target, dtype=np.float32)
    params = np.asarray(params, dtype=np.float32)
    in_maps = make_in_maps(X, target, params)
    if _NC_CACHE is None:
        _NC_CACHE = build_kernel()
    res = run_bass_kernel_spmd(_NC_CACHE, in_maps, core_ids=list(range(NCORES)))
    return combine(res.results, X, target, params)


# revision 3
# speedup vs baseline: 1.1248x; 1.0792x over previous
"""KTA loss v2c: symmetric circulant sharding, s2 on PE chains, exact diag.

Same sharding as v2 (see kernel_v2.py docstring). Key points:
- E = exp(2 Xs_i.Xs_j - sqb16_j + bias_i) in bf16, persisted in SBUF.
- bias_i = -(2 sq_i + nsqb_i) is host-computed so the diagonal entry is
  exp(~1e-6) -> exactly 1.0 in bf16: no on-device diagonal correction at all.
  Host adds sum(t^2) - sum(tb16*t) to s2 to swap the computed diagonal for
  the exact one; s1 needs no adjustment (1.0^2 both ways).
- s1 = sum E^2 via DVE STT accum (32 weighted slots).
- s2 = t^T E t via PE: per 512-col position, a PSUM [1,512] chain over the
  covering strips, stationary = w*t_strip (block weight folded in, bf16);
  rows are copied PSUM->SBUF (ACT/DVE split), bounced through DRAM into a
  [16,512] tile, and reduced against exact-f32 t in one STT.
"""

import numpy as np
import ml_dtypes

import concourse.bass as bass
import concourse.bacc as bacc
import concourse.tile as tile
import concourse.mybir as mybir
from concourse.bass_utils import run_bass_kernel_spmd

N = 8192
D = 64
NCORES = 8
NG = 16
GW = 512
NSTRIP = 8

F32 = mybir.dt.float32
BF16 = mybir.dt.bfloat16
BF = ml_dtypes.bfloat16

ACT_CHUNKS_A = [(0, 1536), (1536, 1536), (3072, 1536)]
ACT_CHUNKS_B = [(4096, 1536), (5632, 1536), (7168, 1024)]
RED_A = [(0, 512, 1.0), (512, 1024, 2.0), (1536, 1536, 2.0), (3072, 1536, 2.0)]
RED_B = [(4096, 512, 1.0), (4608, 1024, 2.0), (5632, 1536, 2.0),
         (7168, 1024, 2.0)]
NRED_A = len(RED_A)
NRED_B = len(RED_B)
NSLOT = 4 * NRED_A + 4 * NRED_B      # 32 s1 slots
ESTRIDE = 4608
# acc columns: [0,32) s1, 32 = s2-chain (partitions 0..9), [33,45) = B-strip
# s2-STT slots (3 pieces x 4 strips), [45,49) = A-strip pos-8 s2-STT slots
ACCW = 56
MOVED_PIECES = [(5120, 512), (5632, 1536), (7168, 1024)]
TCOLP0 = 4096                         # tcolp covers [4096,8192)
NACT_COPIES = 16                      # v-row copies on ACT; rest on DVE
MOVED0 = 5120                         # local col where DVE-handled s2 starts
MOVEDW = 8192 - MOVED0                # 3584 cols, weight 2 blocks only


def _act_chunks(s):
    return ACT_CHUNKS_A if s < 4 else ACT_CHUNKS_B


def _reds(s):
    return RED_A if s < 4 else RED_B


def _slot_base(s):
    return s * NRED_A if s < 4 else 4 * NRED_A + (s - 4) * NRED_B


def _ebase(s):
    return s * ESTRIDE - (0 if s < 4 else 4096)


def build_kernel():
    nc = bacc.Bacc("TRN2", target_bir_lowering=False)

    rt_d = nc.dram_tensor("rt", [D + 1, N], BF16, kind="ExternalInput")
    lt_d = nc.dram_tensor("lt", [D + 1, 1024], BF16, kind="ExternalInput")
    rsqn_d = nc.dram_tensor("rsqn", [128, 2 * NSTRIP], F32, kind="ExternalInput")
    tw_d = nc.dram_tensor("tw", [128, 2 * NSTRIP], BF16, kind="ExternalInput")
    w16_d = nc.dram_tensor("w16", [16, GW], F32, kind="ExternalInput")
    trow_d = nc.dram_tensor("trow", [1, 8192 - TCOLP0], BF16, kind="ExternalInput")
    acc_d = nc.dram_tensor("acc", [128, ACCW], F32, kind="ExternalOutput")
    vs_d = nc.dram_tensor("vs_scratch", [16, GW], F32)

    with tile.TileContext(nc) as tc:
        with (
            tc.tile_pool(name="const", bufs=1) as cpool,
            tc.tile_pool(name="dmyv", bufs=2) as vpool,
            tc.tile_pool(name="mmpool", bufs=2, space="PSUM") as mpool,
            tc.tile_pool(name="trc", bufs=2, space="PSUM") as tpool,
        ):
            R = cpool.tile([D + 1, N], BF16, tag="R")
            L = cpool.tile([D + 1, 1024], BF16, tag="L")
            rsqn = cpool.tile([128, 2 * NSTRIP], F32, tag="rsqn")
            tw = cpool.tile([128, 2 * NSTRIP], BF16, tag="tw")
            w16 = cpool.tile([16, GW], F32, tag="w16")
            vsb = cpool.tile([16, GW], F32, tag="vsb")
            trow = cpool.tile([1, 8192 - TCOLP0], BF16, tag="trow")
            tcolp = cpool.tile([128, 8192 - TCOLP0], BF16, tag="tcolp")
            acc = cpool.tile([128, ACCW], F32, tag="acc")
            Ebuf = cpool.tile([128, NSTRIP * ESTRIDE], BF16, tag="Ebuf")

            # ---- setup: DMAs only, spread across issue queues --------------
            dmaengines = [nc.sync, nc.scalar, nc.sync, nc.gpsimd]
            nc.gpsimd.dma_start(out=L[:, :], in_=lt_d[:, :])
            nc.gpsimd.dma_start(out=rsqn[:, :], in_=rsqn_d[:, :])
            nc.gpsimd.dma_start(out=tw[:, :], in_=tw_d[:, :])
            nc.gpsimd.dma_start(out=w16[:, :], in_=w16_d[:, :])
            nc.gpsimd.dma_start(out=trow[:, :], in_=trow_d[:, :])
            nc.sync.dma_start(out=R[:, 0:512], in_=rt_d[:, 0:512])
            nc.scalar.dma_start(out=R[:, 512:1024], in_=rt_d[:, 512:1024])
            for s8 in range(1, 8):
                sl = slice(s8 * 1024, (s8 + 1) * 1024)
                dmaengines[s8 % 4].dma_start(out=R[:, sl], in_=rt_d[:, sl])
            # Pool builds the column-t tile for the DVE-handled s2 ranges
            for z in range(4):
                zsl = slice(z * 1024, (z + 1) * 1024)
                nc.gpsimd.partition_broadcast(tcolp[:, zsl], trow[:, zsl])

            # ---- strip pass: mm -> exp -> s1 STT on DVE --------------------
            def do_strip(s, pump=None):
                lhsT = L[:, s * 128:(s + 1) * 128]
                biasap = rsqn[:, s:s + 1]
                sbase = _slot_base(s)
                reds = _reds(s)
                eb = _ebase(s)
                ri = 0
                for c0, cw in _act_chunks(s):
                    mm = mpool.tile([128, 1536], F32, tag="mm")
                    for j in range(cw // 512):
                        nc.tensor.matmul(
                            mm[:, j * 512:(j + 1) * 512],
                            lhsT,
                            R[:, c0 + j * 512:c0 + (j + 1) * 512],
                            start=True, stop=True,
                        )
                    nc.scalar.activation(
                        out=Ebuf[:, eb + c0:eb + c0 + cw], in_=mm[:, 0:cw],
                        func=mybir.ActivationFunctionType.Exp,
                        bias=biasap, scale=1.0,
                    )
                    while ri < len(reds) and \
                            reds[ri][0] + reds[ri][1] <= c0 + cw:
                        r0, rw, _w = reds[ri]
                        slot = sbase + ri
                        sc1 = vpool.tile([128, 1536], BF16, tag="sc1")
                        nc.vector.scalar_tensor_tensor(
                            out=sc1[:, 0:rw],
                            in0=Ebuf[:, eb + r0:eb + r0 + rw],
                            scalar=1.0,
                            in1=Ebuf[:, eb + r0:eb + r0 + rw],
                            op0=mybir.AluOpType.mult,
                            op1=mybir.AluOpType.mult,
                            accum_out=acc[:, slot:slot + 1],
                        )
                        ri += 1
                    if s >= 4 and c0 + cw > MOVED0:
                        # moved-s2 piece of this chunk on DVE (weight 2)
                        p0 = max(c0, MOVED0)
                        pw = c0 + cw - p0
                        pi = MOVED_PIECES.index((p0, pw))
                        sc2 = vpool.tile([128, 1536], BF16, tag="sc2")
                        nc.vector.scalar_tensor_tensor(
                            out=sc2[:, 0:pw],
                            in0=Ebuf[:, eb + p0:eb + p0 + pw],
                            scalar=1.0,
                            in1=tcolp[:, p0 - TCOLP0:p0 - TCOLP0 + pw],
                            op0=mybir.AluOpType.mult,
                            op1=mybir.AluOpType.mult,
                            accum_out=acc[:, 33 + 3 * (s - 4) + pi:
                                          34 + 3 * (s - 4) + pi],
                        )
                    if pump is not None:
                        pump()

            # ---- s2 position chains on PE ----------------------------------
            def covering(pos):
                out = []
                if pos <= 8:
                    for s in range(4):
                        out.append((s, 2 * s if pos == 0 else 2 * s + 1))
                if pos in (8, 9):
                    for s in range(4, 8):
                        out.append((s, 2 * s if pos == 8 else 2 * s + 1))
                return out

            ncopies = [0]

            def do_pos(pos, vstage):
                v = tpool.tile([1, GW], F32, tag="v")
                cov = covering(pos)
                for k, (s, twcol) in enumerate(cov):
                    e0 = _ebase(s) + pos * GW
                    nc.tensor.matmul(
                        v[:, :],
                        tw[:, twcol:twcol + 1],
                        Ebuf[:, e0:e0 + GW],
                        start=(k == 0), stop=(k == len(cov) - 1),
                    )
                # PSUM -> SBUF (split ACT/DVE), then through DRAM onto
                # partition `pos` of vsb
                if ncopies[0] < NACT_COPIES:
                    nc.scalar.copy(out=vstage[:, :], in_=v[:, :])
                else:
                    nc.vector.tensor_copy(out=vstage[:, :], in_=v[:, :])
                dmae = [nc.sync, nc.gpsimd][ncopies[0] % 2]
                ncopies[0] += 1
                dmae.dma_start(
                    out=bass.AP(tensor=vs_d, offset=pos * GW,
                                ap=[[0, 1], [1, GW]]),
                    in_=vstage[:, :])

            vstages = [cpool.tile([1, GW], F32, tag=f"vst{i}", name=f"vst{i}")
                       for i in range(4)]
            for s in range(4):
                do_strip(s)
            pstate = {"emitted": 0, "chunk": 0}

            def pump():
                pstate["chunk"] += 1
                want = (pstate["chunk"] * 8) // 12
                while pstate["emitted"] < min(want, 8):
                    p = pstate["emitted"]
                    do_pos(p, vstages[p % 4])
                    pstate["emitted"] += 1

            for s in range(4, 8):
                do_strip(s, pump=pump)
            while pstate["emitted"] < 8:
                p = pstate["emitted"]
                do_pos(p, vstages[p % 4])
                pstate["emitted"] += 1
            do_pos(8, vstages[0])
            do_pos(9, vstages[1])

            # one gather DMA for the 9 chained position rows
            nc.gpsimd.dma_start(
                out=vsb[0:10, :],
                in_=bass.AP(tensor=vs_d, offset=0, ap=[[GW, 10], [1, GW]]))
            scf = vpool.tile([16, GW], F32, tag="scf")
            nc.vector.scalar_tensor_tensor(
                out=scf[0:10, :], in0=vsb[0:10, :], scalar=1.0, in1=w16[0:10, :],
                op0=mybir.AluOpType.mult, op1=mybir.AluOpType.mult,
                accum_out=acc[0:10, 32:33],
            )

            nc.sync.dma_start(out=acc_d[:, :], in_=acc[:, :])

    nc.compile()
    return nc


_NC_CACHE = None


def _prep(X, target, params):
    Xs = X.astype(np.float64) * np.sqrt(params.astype(np.float64))
    Xsb = Xs.astype(BF)
    sq = (Xsb.astype(np.float64) ** 2).sum(axis=1)
    tb = target.astype(BF)
    return Xsb, sq, tb


def _perm(c):
    return np.concatenate(
        [np.arange(((c + g) % NG) * GW, ((c + g) % NG) * GW + GW)
         for g in range(NG)])


def make_in_maps(X, target, params):
    X = np.asarray(X, dtype=np.float32)
    target = np.asarray(target, dtype=np.float32)
    params = np.asarray(params, dtype=np.float32)
    Xsb, sq, tb = _prep(X, target, params)
    XsbT = np.ascontiguousarray(Xsb.T)
    L2T = np.ascontiguousarray((2.0 * Xsb.astype(np.float64)).astype(BF).T)
    nsqb = (-sq).astype(BF)
    nsqb64 = nsqb.astype(np.float64)
    # bias_i = -(2 sq_i + nsqb_i): diagonal mm + bias == ~0 -> E_ii == 1.0
    bias = -(2.0 * sq + nsqb64)
    in_maps = []
    for c in range(NCORES):
        perm = _perm(c)
        rows = np.concatenate([perm[:GW], perm[8 * GW:9 * GW]])
        rt = np.vstack([XsbT[:, perm], nsqb[perm][None, :]])
        lt = np.vstack([L2T[:, rows], np.ones((1, 1024), dtype=BF)])
        b1 = bias[rows].astype(np.float32).reshape(NSTRIP, 128).T
        b2 = (2.0 * bias[rows]).astype(np.float32).reshape(NSTRIP, 128).T
        rsqn = np.ascontiguousarray(np.hstack([b1, b2]))
        tstrips = tb[rows].reshape(NSTRIP, 128)
        tw = np.zeros((128, 2 * NSTRIP), dtype=BF)
        for s in range(NSTRIP):
            tw[:, 2 * s] = tstrips[s]
            tw[:, 2 * s + 1] = (2.0 * tstrips[s].astype(np.float64)).astype(BF)
        w16 = np.ascontiguousarray(
            target[perm].astype(np.float32).reshape(16, GW))
        trow = np.ascontiguousarray(tb[perm][TCOLP0:8192][None, :])
        in_maps.append({
            "rt": np.ascontiguousarray(rt),
            "lt": np.ascontiguousarray(lt),
            "rsqn": rsqn,
            "tw": tw,
            "w16": w16,
            "trow": trow,
        })
    return in_maps


def combine(results, X, target, params):
    target = np.asarray(target, np.float32)
    t64 = target.astype(np.float64)
    tb64 = target.astype(BF).astype(np.float64)
    s1 = 0.0
    s2 = 0.0
    for c in range(NCORES):
        acc = results[c]["acc"].astype(np.float64)
        for s in range(NSTRIP):
            sbase = _slot_base(s)
            for ri, (_r0, _rw, w) in enumerate(_reds(s)):
                s1 += w * acc[:, sbase + ri].sum()
        s2 += acc[0:10, 32].sum()
        perm = _perm(c)
        rows = np.concatenate([perm[:GW], perm[8 * GW:9 * GW]])
        trows = t64[rows].reshape(NSTRIP, 128)
        for s in range(4, NSTRIP):
            for pi in range(3):
                s2 += 2.0 * (trows[s] * acc[:, 33 + 3 * (s - 4) + pi]).sum()

    # swap computed diagonal (E_ii == 1.0, tb16_i * t_i) for the exact one
    s2 += (t64 * t64).sum() - (tb64 * t64).sum()
    return np.float32(-s2 / (N * np.sqrt(s1)))


def kernel(X, target, params):
    global _NC_CACHE
    X = np.asarray(X, dtype=np.float32)
    target = np.asarray(target, dtype=np.float32)
    params = np.asarray(params, dtype=np.float32)
    in_maps = make_in_maps(X, target, params)
    if _NC_CACHE is None:
        _NC_CACHE = build_kernel()
    res = run_bass_kernel_spmd(_NC_CACHE, in_maps, core_ids=list(range(NCORES)))
    return combine(res.results, X, target, params)
